# revision 22
# baseline (speedup 1.0000x reference)
"""Trainium2 Bass kernel for MinibatchDiscrimination.

Reference computation:
    M    = einsum('bi,iok->bok', x, T)            # [B, OUT, KD]
    norm = |M[None,:] - M[:,None]|.sum(axis=3)    # [B, B, OUT] pairwise L1 over KD
    o_b  = exp(-norm).sum(axis=0) - 1             # [B, OUT]
    out  = concat([x, o_b], axis=1)               # [B, IN+OUT]

Sharding: the B (row) axis of the pairwise interaction is sharded across
the 8 cores (32 j-rows each); every core computes the full M from
replicated x/T inputs (no collectives since full inputs are staged
per-core).

Device layout: everything lives as [(o,k)-partition, i-free] tiles, where
(o,k) flat index = 8*o + k, split into 16 groups g of 128 partitions.

The pairwise L1 uses the relu identity  sum_k |d_k| = 2*sum_k relu(d_k)
- sum_k d_k  so the elementwise stage is ONE fused DVE
tensor_scalar(op0=subtract, op1=max, scalar2=0) per (j, group) — the
single-source 4x-eligible op.  The remaining pieces:
  - k-sum     : PE one-hot matmuls.  S32[p, h, m] selects partition groups
                of 8 into 16 columns; group pairs (2q, 2q+1) accumulate
                into the 32-row PSUM region at col_grp 32q, so partition p
                of PSUM tile t holds o = 128*t + p.
  - -0.5*sum_k M[..] correction (i-dependent): precomputed once as
                SMnh = sum_S(-0.5*M) (bf16 half-scaling is exact), then
                delivered into each (j, t) PSUM accumulation by a single
                fp32 identity matmul preload.
  - +sum_k mj  correction (partition-only): folded into the ACT exp bias:
                exp(-2*PSUM + bias), bias = sum_k(-mj) per partition.
  - exp + i-sum: one ACT activation(Exp, scale=-2, bias=negSMj col,
                accum_out=...) per (j, tile): accum gives sum_i exp(-L1),
                a column of o_b^T.
  - self-term : all transforms between the two reductions of the
                bitwise-identical M / Mj columns are exact in fp32
                (negation, x0.5, x(-2) commute with rounding), so
                L1[i=j] is exactly +0.0; we subtract the kernel's own
                exp(+0.0) value (c_col) instead of the constant 1.0 so
                the cancellation is exact regardless of the HW exp
                spline.

bf16 is used for M / relu-diff tiles: the minimum off-diagonal norm is
~24.7, so every off-diagonal exp term is < 2e-11 and is absorbed by the
~1.0 self-term in the fp32 accumulation regardless of ~O(1) bf16 error
in the norm; the result is bit-identical to the fp32 reference (exact
zeros).
"""

import sys

import numpy as np

for _p in ("/opt/trn_rl_repo",):
    if _p not in sys.path:
        sys.path.insert(0, _p)

import ml_dtypes

B = 256          # batch
IN = 2048        # in_features
OUT = 256        # out_features
KD = 8           # kernel_dims
NCORES = 8
JB = B // NCORES  # 32 j-rows per core
OK = OUT * KD     # 2048 (o,k) columns, k fastest
G = OK // 128     # 16 (o,k)-groups of 128 partitions
KC = IN // 128    # 16 contraction chunks of 128

_CACHE = {}


def _build_nc():
    import concourse.bacc as bacc
    import concourse.mybir as mybir
    import concourse.tile as tile

    dt = mybir.dt
    alu = mybir.AluOpType
    act = mybir.ActivationFunctionType

    # Bacc (not plain Bass): its finalize() runs compile(), which includes
    # move_matmul_waits_to_ldweights + generate_event_semaphores — the
    # passes that legalize multi-wait instructions for walrus.
    nc = bacc.Bacc()

    xTs_d = nc.declare_dram_parameter("xTs", [128, KC, B], dt.bfloat16, isOutput=False)
    xjTs_d = nc.declare_dram_parameter("xjTs", [128, KC, JB], dt.bfloat16, isOutput=False)
    Tst_d = nc.declare_dram_parameter("Tst", [G, 128, KC, 128], dt.bfloat16, isOutput=False)
    S32_d = nc.declare_dram_parameter("S32", [128, 2, 32], dt.bfloat16, isOutput=False)
    Sneg_d = nc.declare_dram_parameter("Sneg32", [128, 2, 32], dt.bfloat16, isOutput=False)
    Ident_d = nc.declare_dram_parameter("Ident", [128, 128], dt.float32, isOutput=False)
    ob_d = nc.declare_dram_parameter("ob", [2, 128, JB], dt.float32, isOutput=True)

    with tile.TileContext(nc) as tc:
        with (
            tc.tile_pool(name="const", bufs=1) as constp,
            tc.tile_pool(name="mtiles", bufs=1) as mpool,
            tc.tile_pool(name="tw", bufs=16) as twp,
            tc.tile_pool(name="work", bufs=8) as workp,
            tc.tile_pool(name="escr", bufs=2) as escrp,
            tc.tile_pool(name="outp", bufs=1) as outp,
        ):
            # ---- constants / global loads ----
            S32 = constp.tile([128, 2, 32], dt.bfloat16, tag="s32")
            nc.sync.dma_start(S32[:], S32_d[:])
            Sneg = constp.tile([128, 2, 32], dt.bfloat16, tag="sneg")
            nc.sync.dma_start(Sneg[:], Sneg_d[:])
            Ident = constp.tile([128, 128], dt.float32, tag="ident")
            nc.sync.dma_start(Ident[:], Ident_d[:])
            xT = constp.tile([128, KC, B], dt.bfloat16, tag="xT")
            nc.sync.dma_start(xT[:], xTs_d[:])
            xjT = constp.tile([128, KC, JB], dt.bfloat16, tag="xjT")
            nc.sync.dma_start(xjT[:], xjTs_d[:])

            # self-term constant c = exp(+0.0) computed with the same
            # func/scale structure as the hot loop so it cancels exactly.
            czero = constp.tile([128, 1], dt.float32, tag="czero")
            nc.vector.memset(czero[:], 0.0)
            c_col = constp.tile([128, 1], dt.float32, tag="ccol")
            nc.scalar.activation(c_col[:], czero[:], act.Exp, scale=-2.0)

            # phase-1 and phase-2 PSUM pools stay open together with
            # disjoint banks (2 tags x 2 bufs each = 8 banks total).
            phase1 = tc.tile_pool(name="psum_m", bufs=2, space="PSUM")
            psmp = phase1.__enter__()
            pslp_cm = tc.tile_pool(name="psum_l1", bufs=2, space="PSUM")
            pslp = pslp_cm.__enter__()

            # ---- phase 1: M = x @ T2d in [(o,k), i] layout, bf16 ----
            m_sb = []    # 16 x [128, B] bf16
            mnh_sb = []  # 16 x [128, B] bf16, exactly -0.5 * m_sb
            mjb_sb = []  # 16 x [128, JB] bf16
            mj_sb = []   # 16 x [128, JB] fp32 (same bf16-rounded values)
            for g in range(G):
                tw = twp.tile([128, KC, 128], dt.bfloat16, tag="tw")
                nc.sync.dma_start(tw[:], Tst_d[g])
                ps_m = psmp.tile([128, B], dt.float32, tag="psm")
                ps_j = psmp.tile([128, JB], dt.float32, tag="psj")
                for kc in range(KC):
                    nc.tensor.matmul(
                        ps_m[:], tw[:, kc, :], xT[:, kc, :],
                        start=(kc == 0), stop=(kc == KC - 1),
                    )
                for kc in range(KC):
                    nc.tensor.matmul(
                        ps_j[:], tw[:, kc, :], xjT[:, kc, :],
                        start=(kc == 0), stop=(kc == KC - 1),
                    )
                # all PSUM->bf16 copies on ACT so rounding is engine-uniform
                mg = mpool.tile([128, B], dt.bfloat16, tag=f"m{g}")
                nc.scalar.activation(mg[:], ps_m[:], act.Copy)
                mnh = mpool.tile([128, B], dt.bfloat16, tag=f"mnh{g}")
                nc.scalar.activation(mnh[:], ps_m[:], act.Copy, scale=-0.5)
                mjb = mpool.tile([128, JB], dt.bfloat16, tag=f"mjb{g}")
                nc.scalar.activation(mjb[:], ps_j[:], act.Copy)
                # exact fp32 upcast of the bf16-rounded values (the ts
                # scalar operand must be fp32 and must equal mg's column
                # values bitwise for the self-term to cancel)
                mjg = mpool.tile([128, JB], dt.float32, tag=f"mj{g}")
                nc.vector.tensor_copy(mjg[:], mjb[:])
                m_sb.append(mg)
                mnh_sb.append(mnh)
                mjb_sb.append(mjb)
                mj_sb.append(mjg)

            # ---- phase 1b: j-independent corrections ----
            # SMnh[t] = sum_S(-0.5*M) : [128, B] fp32, o = 128t + p
            # negSMj[t] = sum_S(-mj)  : [128, JB] fp32
            smnh_sb = []
            negsmj_sb = []
            for t in range(2):
                ps_nh = psmp.tile([128, B], dt.float32, tag="psm")
                ps_sj = psmp.tile([128, JB], dt.float32, tag="psj")
                for g in range(8 * t, 8 * t + 8):
                    q, h = (g % 8) // 2, g % 2
                    nc.tensor.matmul(
                        ps_nh[32 * q : 32 * q + 32, :],
                        S32[:, h, :], mnh_sb[g][:],
                        start=(h == 0), stop=(h == 1),
                        tile_position=(0, 32 * q),
                        skip_group_check=True,
                    )
                    nc.tensor.matmul(
                        ps_sj[32 * q : 32 * q + 32, :],
                        Sneg[:, h, :], mjb_sb[g][:],
                        start=(h == 0), stop=(h == 1),
                        tile_position=(0, 32 * q),
                        skip_group_check=True,
                    )
                smnh = constp.tile([128, B], dt.float32, tag=f"smnh{t}")
                nc.vector.tensor_copy(smnh[:], ps_nh[:])
                negsmj = constp.tile([128, JB], dt.float32, tag=f"negsmj{t}")
                nc.vector.tensor_copy(negsmj[:], ps_sj[:])
                smnh_sb.append(smnh)
                negsmj_sb.append(negsmj)

            # ---- phase 2: relu-diff, k-sum, exp, i-sum ----
            acc = []
            for t in range(2):
                a = constp.tile([128, JB], dt.float32, tag=f"acc{t}")
                acc.append(a)

            for jj in range(JB):
                ps_l1_0 = pslp.tile([128, B], dt.float32, tag="l1_0")
                ps_l1_1 = pslp.tile([128, B], dt.float32, tag="l1_1")
                ps_l1 = [ps_l1_0, ps_l1_1]
                # fp32 identity matmul preloads the -0.5*sum_k M correction
                for t in range(2):
                    nc.tensor.matmul(
                        ps_l1[t][:], Ident[:], smnh_sb[t][:],
                        start=True, stop=False, skip_group_check=True,
                    )
                # evens-then-odds group order: consecutive one-hot matmuls
                # hit different PSUM col-groups and can overlap on the PE
                for t in range(2):
                    for g in [0, 2, 4, 6, 1, 3, 5, 7]:
                        gg = 8 * t + g
                        rl = workp.tile([128, B], dt.bfloat16, tag="relu")
                        nc.vector.tensor_scalar(
                            out=rl[:],
                            in0=m_sb[gg][:],
                            scalar1=mj_sb[gg][:, jj : jj + 1],
                            scalar2=0.0,
                            op0=alu.subtract,
                            op1=alu.max,
                        )
                        q, h = g // 2, g % 2
                        nc.tensor.matmul(
                            ps_l1[t][32 * q : 32 * q + 32, :],
                            S32[:, h, :], rl[:],
                            start=False, stop=(h == 1),
                            tile_position=(0, 32 * q),
                            skip_group_check=True,
                        )
                for t in range(2):
                    e_scr = escrp.tile([128, B], dt.float32, tag="escr")
                    nc.scalar.activation(
                        e_scr[:], ps_l1[t][:], act.Exp,
                        scale=-2.0,
                        bias=negsmj_sb[t][:, jj : jj + 1],
                        accum_out=acc[t][:, jj : jj + 1],
                    )

            pslp_cm.__exit__(None, None, None)
            phase1.__exit__(None, None, None)

            # ---- subtract self-term, store ----
            for t in range(2):
                ob_t = outp.tile([128, JB], dt.float32, tag=f"ob{t}")
                nc.vector.tensor_tensor(
                    out=ob_t[:],
                    in0=acc[t][:],
                    in1=c_col[:].broadcast_to([128, JB]),
                    op=alu.subtract,
                )
                nc.sync.dma_start(ob_d[t], ob_t[:])

    if not nc.is_finalized():
        nc.finalize()
    return nc


def _prep_inputs(x, T):
    bf16 = ml_dtypes.bfloat16
    xb = x.astype(bf16)                      # [B, IN]
    T2b = T.reshape(IN, OK).astype(bf16)     # [IN, (o,k)] k fastest

    # xTs[p, kc, i] = x[i, 128*kc + p]
    xTs = np.ascontiguousarray(
        xb.T.reshape(KC, 128, B).transpose(1, 0, 2)
    )
    # Tst[g, p, kc, c] = T2b[128*kc + p, 128*g + c]
    Tst = np.ascontiguousarray(
        T2b.reshape(KC, 128, G, 128).transpose(2, 1, 0, 3)
    )
    S32 = np.zeros((128, 2, 32), dtype=bf16)
    p = np.arange(128)
    S32[p, 0, p // 8] = 1
    S32[p, 1, 16 + p // 8] = 1
    Sneg32 = (-S32.astype(np.float32)).astype(bf16)
    Ident = np.eye(128, dtype=np.float32)
    return xTs, Tst, S32, Sneg32, Ident


def kernel(x, T):
    from concourse.bass_utils import run_bass_kernel_spmd

    x = np.asarray(x)
    T = np.asarray(T)

    if "nc" not in _CACHE:
        _CACHE["nc"] = _build_nc()
    nc = _CACHE["nc"]

    xTs, Tst, S32, Sneg32, Ident = _prep_inputs(x, T)
    in_maps = []
    for c in range(NCORES):
        in_maps.append(
            {
                "xTs": xTs,
                "xjTs": np.ascontiguousarray(xTs[:, :, c * JB : (c + 1) * JB]),
                "Tst": Tst,
                "S32": S32,
                "Sneg32": Sneg32,
                "Ident": Ident,
            }
        )

    res = run_bass_kernel_spmd(nc, in_maps, list(range(NCORES)))

    o_b = np.empty((B, OUT), dtype=np.float32)
    for c in range(NCORES):
        ob_c = np.asarray(res.results[c]["ob"])  # [2, 128, JB]
        for t in range(2):
            o_b[c * JB : (c + 1) * JB, t * 128 : (t + 1) * 128] = ob_c[t].T

    return np.concatenate([x.astype(np.float32), o_b], axis=1)


# revision 34
# speedup vs baseline: 23199.2493x; 23199.2493x over previous
"""Trainium2 Bass kernel for MinibatchDiscrimination.

Reference computation:
    M    = einsum('bi,iok->bok', x, T)            # [B, OUT, KD]
    norm = |M[None,:] - M[:,None]|.sum(axis=3)    # [B, B, OUT] pairwise L1 over KD
    o_b  = exp(-norm).sum(axis=0) - 1             # [B, OUT]
    out  = concat([x, o_b], axis=1)               # [B, IN+OUT]

Sharding (8 cores, no collectives): 2-way split of the OUT axis x 4-way
split of the B (j-row) axis.  Core c = 4*s + r computes o_b for j-block r
(64 rows) and o-half s (128 outs).  Each core computes M only for its
o-half from replicated x plus its own T-half — full inputs are staged
per-core so no device-to-device communication is needed.

Device layout: [(o,k)-partition, i-free]; the core's 1024 (o,k) columns
split into 8 groups g of 128 partitions (o_local = 16g + m, k = p%8).

The pairwise L1 uses the relu identity  sum_k |d_k| = 2*sum_k relu(d_k)
- sum_k d_k  so the elementwise stage is ONE fused DVE
tensor_scalar(op0=subtract, op1=max, scalar2=0) per (j, group) — the
single-source 4x-eligible op.  The remaining pieces:
  - k-sum     : PE one-hot matmuls.  S32[p, h, m] selects partition
                groups of 8 into 16 columns; group pairs (2q, 2q+1)
                accumulate into the 32-row PSUM region at col_grp 32q, so
                PSUM partition p holds o_local = p.
  - -0.5*sum_k M (i-dependent correction): precomputed TRANSPOSED
                (SMnhT[i, o] = sum_S(-0.5*M), bf16 half-scaling exact,
                via one-hot matmuls with M as lhsT), then delivered into
                each j's PSUM accumulation by two fp32 transpose-mode
                matmuls (2 cyc/row vs 4 for a plain fp32 matmul).
  - +sum_k mj correction (partition-only): folded into the ACT exp bias:
                exp(-2*PSUM + bias), bias = sum_k(-mj) per partition.
  - exp + i-sum: one ACT activation(Exp, scale=-2, bias=negSMj col,
                accum_out=...) per j: accum gives sum_i exp(-L1), a
                column of o_b^T.
  - self-term : M and Mj columns come from the SAME matmul (the rhs is
                [xT | xjT] merged, 320 columns), so they are bitwise
                equal; every transform between the two reduction paths
                is exact in fp32 (negation, x0.5, x(-2), x1.0 products
                commute with rounding), so L1[i=j] is exactly +0.0.  We
                subtract the kernel's own exp(+0.0) value (c_col) instead
                of the constant 1.0 so the cancellation is exact
                regardless of the HW exp spline.

bf16 is used for M / relu tiles: the minimum off-diagonal norm is ~24.7,
so every off-diagonal exp term is < 2e-11 and is absorbed by the ~1.0
self-term in the fp32 accumulation regardless of ~O(1) bf16 error in the
norm; the result is bit-identical to the fp32 reference (exact zeros).
"""

import sys

import numpy as np

for _p in ("/opt/trn_rl_repo",):
    if _p not in sys.path:
        sys.path.insert(0, _p)

import ml_dtypes

B = 256          # batch
IN = 2048        # in_features
OUT = 256        # out_features
KD = 8           # kernel_dims
NCORES = 8
OSPLIT = 2       # o-half per core
JSPLIT = 4       # j-blocks
JBLK = B // JSPLIT        # 64 j-rows per core
OH = OUT // OSPLIT        # 128 outs per core
G = OH * KD // 128        # 8 (o,k)-groups of 128 partitions per core
KC = IN // 128            # 16 contraction chunks of 128
RW = B + JBLK             # 320: merged [xT | xjT] rhs columns

_CACHE = {}


def _build_nc():
    import concourse.bacc as bacc
    import concourse.mybir as mybir
    import concourse.tile as tile

    dt = mybir.dt
    alu = mybir.AluOpType
    act = mybir.ActivationFunctionType

    # Bacc (not plain Bass): its finalize() runs compile(), which includes
    # move_matmul_waits_to_ldweights + generate_event_semaphores — the
    # passes that legalize multi-wait instructions for walrus.
    nc = bacc.Bacc()

    xA_d = nc.declare_dram_parameter("xAll", [128, KC, RW], dt.bfloat16, isOutput=False)
    Tst_d = nc.declare_dram_parameter("Tst", [G, 128, KC, 128], dt.bfloat16, isOutput=False)
    S32_d = nc.declare_dram_parameter("S32", [128, 2, 32], dt.bfloat16, isOutput=False)
    Sneg_d = nc.declare_dram_parameter("Sneg32", [128, 2, 32], dt.bfloat16, isOutput=False)
    Ident_d = nc.declare_dram_parameter("Ident", [128, 128], dt.float32, isOutput=False)
    S128_d = nc.declare_dram_parameter("S128", [128, G, 128], dt.bfloat16, isOutput=False)
    ob_d = nc.declare_dram_parameter("ob", [128, JBLK], dt.float32, isOutput=True)

    with tile.TileContext(nc) as tc:
        with (
            tc.tile_pool(name="const", bufs=1) as constp,
            tc.tile_pool(name="mtiles", bufs=1) as mpool,
            tc.tile_pool(name="tw", bufs=G) as twp,
            tc.tile_pool(name="work", bufs=1) as workp,
            tc.tile_pool(name="escr", bufs=1) as escrp,
            tc.tile_pool(name="outp", bufs=1) as outp,
        ):
            # ---- constants / global loads ----
            S32 = constp.tile([128, 2, 32], dt.bfloat16, tag="s32")
            nc.sync.dma_start(S32[:], S32_d[:])
            Sneg = constp.tile([128, 2, 32], dt.bfloat16, tag="sneg")
            nc.sync.dma_start(Sneg[:], Sneg_d[:])
            Ident = constp.tile([128, 128], dt.float32, tag="ident")
            nc.sync.dma_start(Ident[:], Ident_d[:])
            S128 = constp.tile([128, G, 128], dt.bfloat16, tag="s128")
            nc.sync.dma_start(S128[:], S128_d[:])
            xA = constp.tile([128, KC, RW], dt.bfloat16, tag="xA")
            nc.sync.dma_start(xA[:], xA_d[:])

            # self-term constant c = exp(+0.0) computed with the same
            # func/scale structure as the hot loop so it cancels exactly.
            czero = constp.tile([128, 1], dt.float32, tag="czero")
            nc.vector.memset(czero[:], 0.0)
            c_col = constp.tile([128, 1], dt.float32, tag="ccol")
            nc.scalar.activation(c_col[:], czero[:], act.Exp, scale=-2.0)

            phase1 = tc.tile_pool(name="psum_m", bufs=2, space="PSUM")
            psmp = phase1.__enter__()
            pslp_cm = tc.tile_pool(name="psum_l1", bufs=1, space="PSUM")
            pslp = pslp_cm.__enter__()

            # ---- phase 1: M(+Mj) = x @ T-half in [(o,k), i|j] layout ----
            m_sb = []    # G x [128, B] bf16
            mnh_sb = []  # G x [128, B] bf16, exactly -0.5 * m_sb
            mjb_sb = []  # G x [128, JBLK] bf16 (columns 256:320 of ps_m)
            mj_sb = []   # G x [128, JBLK] fp32 (same bf16-rounded values)
            for g in range(G):
                tw = twp.tile([128, KC, 128], dt.bfloat16, tag="tw")
                nc.sync.dma_start(tw[:], Tst_d[g])
                ps_m = psmp.tile([128, RW], dt.float32, tag="psm")
                for kc in range(KC):
                    nc.tensor.matmul(
                        ps_m[:], tw[:, kc, :], xA[:, kc, :],
                        start=(kc == 0), stop=(kc == KC - 1),
                    )
                # all PSUM->bf16 copies on ACT so rounding is engine-uniform
                mg = mpool.tile([128, B], dt.bfloat16, tag=f"m{g}")
                nc.scalar.activation(mg[:], ps_m[:, 0:B], act.Copy)
                mnh = mpool.tile([128, B], dt.bfloat16, tag=f"mnh{g}")
                nc.scalar.activation(mnh[:], ps_m[:, 0:B], act.Copy, scale=-0.5)
                mjb = mpool.tile([128, JBLK], dt.bfloat16, tag=f"mjb{g}")
                nc.scalar.activation(mjb[:], ps_m[:, B:RW], act.Copy)
                # exact fp32 upcast of the bf16-rounded values (the ts
                # scalar operand must be fp32 and must equal mg's column
                # values bitwise for the self-term to cancel)
                mjg = mpool.tile([128, JBLK], dt.float32, tag=f"mj{g}")
                nc.vector.tensor_copy(mjg[:], mjb[:])
                m_sb.append(mg)
                mnh_sb.append(mnh)
                mjb_sb.append(mjb)
                mj_sb.append(mjg)

            # ---- phase 1b: j-independent corrections ----
            # SMnhT[ihalf] [128 i, 128 o_local] fp32 = sum_S(-0.5*M)^T,
            # via one-hot matmuls with Mnh as lhsT (contract partitions):
            #   out[i, m] = sum_p Mnh[p, i] * S32[p, h, m]
            # negSMj [128, JBLK] fp32 = sum_S(-mj)
            smnhT_sb = []
            ps_sj = psmp.tile([128, JBLK], dt.float32, tag="psm")
            for ih in range(2):
                ps_t = pslp.tile([128, B], dt.float32, tag="l1")
                for g in range(G):
                    nc.tensor.matmul(
                        ps_t[:, 0:128],
                        mnh_sb[g][:, 128 * ih : 128 * ih + 128],
                        S128[:, g, :],
                        start=(g == 0), stop=(g == G - 1),
                        skip_group_check=True,
                    )
                smnhT = constp.tile([128, 128], dt.float32, tag=f"smnhT{ih}")
                nc.vector.tensor_copy(smnhT[:], ps_t[:, 0:128])
                smnhT_sb.append(smnhT)
            for g in range(G):
                q, h = g // 2, g % 2
                nc.tensor.matmul(
                    ps_sj[32 * q : 32 * q + 32, :],
                    Sneg[:, h, :], mjb_sb[g][:],
                    start=(h == 0), stop=(h == 1),
                    tile_position=(0, 32 * q),
                    skip_group_check=True,
                )
            negsmj = constp.tile([128, JBLK], dt.float32, tag="negsmj")
            nc.vector.tensor_copy(negsmj[:], ps_sj[:])

            # ---- phase 2: relu-diff, k-sum, exp, i-sum ----
            # tiles are allocated ONCE and rotated manually — per-instance
            # pool tiles would each emit a release + event-semaphore in the
            # kernel tail (~70ns x 576 instances on the DVE queue).
            acc = constp.tile([128, JBLK], dt.float32, tag="acc")
            rl_t = []
            for g in range(G):
                for par in range(2):
                    r = workp.tile([128, B], dt.bfloat16, tag=f"rl{g}_{par}")
                    rl_t.append(r)
            es_t = []
            ps_t2 = []
            for par in range(4):
                e = escrp.tile([128, B], dt.float32, tag=f"escr{par}")
                es_t.append(e)
                p4 = pslp.tile([128, B], dt.float32, tag=f"l1_{par}")
                ps_t2.append(p4)

            for jj in range(JBLK):
                ps_l1 = ps_t2[jj % 4]
                # transpose-mode fp32 matmuls preload the -0.5*sum_k M
                # correction (out = lhsT.T, 2 cyc/row).  The first start=True
                # marks the whole 2KB PSUM bank pending-zero; the second
                # writes its (still-pending) half with start=False.
                for ih in range(2):
                    nc.tensor.matmul(
                        ps_l1[:, 128 * ih : 128 * ih + 128],
                        smnhT_sb[ih][:], Ident[:],
                        is_transpose=True,
                        start=(ih == 0), stop=False, skip_group_check=True,
                    )
                # evens-then-odds group order: consecutive one-hot matmuls
                # hit different PSUM col-groups and can overlap on the PE
                for g in [0, 2, 4, 6, 1, 3, 5, 7]:
                    rl = rl_t[2 * g + (jj % 2)]
                    nc.vector.tensor_scalar(
                        out=rl[:],
                        in0=m_sb[g][:],
                        scalar1=mj_sb[g][:, jj : jj + 1],
                        scalar2=0.0,
                        op0=alu.subtract,
                        op1=alu.max,
                    )
                    q, h = g // 2, g % 2
                    nc.tensor.matmul(
                        ps_l1[32 * q : 32 * q + 32, :],
                        S32[:, h, :], rl[:],
                        start=False, stop=(h == 1),
                        tile_position=(0, 32 * q),
                        skip_group_check=True,
                    )
                nc.scalar.activation(
                    es_t[jj % 4][:], ps_l1[:], act.Exp,
                    scale=-2.0,
                    bias=negsmj[:, jj : jj + 1],
                    accum_out=acc[:, jj : jj + 1],
                )

            pslp_cm.__exit__(None, None, None)
            phase1.__exit__(None, None, None)

            # ---- subtract self-term, store ----
            ob_t = outp.tile([128, JBLK], dt.float32, tag="ob")
            nc.vector.tensor_tensor(
                out=ob_t[:],
                in0=acc[:],
                in1=c_col[:].broadcast_to([128, JBLK]),
                op=alu.subtract,
            )
            nc.sync.dma_start(ob_d[:], ob_t[:])

    if not nc.is_finalized():
        nc.finalize()
    return nc


def _prep_inputs(x, T):
    bf16 = ml_dtypes.bfloat16
    xb = x.astype(bf16)                      # [B, IN]
    T2b = T.reshape(IN, OUT * KD).astype(bf16)

    # xTs[p, kc, i] = x[i, 128*kc + p]
    xTs = np.ascontiguousarray(
        xb.T.reshape(KC, 128, B).transpose(1, 0, 2)
    )
    # TstF[gfull, p, kc, c] = T2b[128*kc + p, 128*gfull + c], gfull 0..15
    TstF = np.ascontiguousarray(
        T2b.reshape(KC, 128, 2 * G, 128).transpose(2, 1, 0, 3)
    )
    S32 = np.zeros((128, 2, 32), dtype=bf16)
    p = np.arange(128)
    S32[p, 0, p // 8] = 1
    S32[p, 1, 16 + p // 8] = 1
    Sneg32 = (-S32.astype(np.float32)).astype(bf16)
    Ident = np.eye(128, dtype=np.float32)
    S128 = np.zeros((128, G, 128), dtype=bf16)
    for g in range(G):
        S128[p, g, 16 * g + p // 8] = 1
    return xTs, TstF, S32, Sneg32, Ident, S128


def kernel(x, T):
    from concourse.bass_utils import run_bass_kernel_spmd

    x = np.asarray(x)
    T = np.asarray(T)

    if "nc" not in _CACHE:
        _CACHE["nc"] = _build_nc()
    nc = _CACHE["nc"]

    xTs, TstF, S32, Sneg32, Ident, S128 = _prep_inputs(x, T)
    in_maps = []
    for c in range(NCORES):
        s, r = divmod(c, JSPLIT)
        xAll = np.concatenate(
            [xTs, xTs[:, :, r * JBLK : (r + 1) * JBLK]], axis=2
        )
        in_maps.append(
            {
                "xAll": np.ascontiguousarray(xAll),
                "Tst": np.ascontiguousarray(TstF[s * G : (s + 1) * G]),
                "S32": S32,
                "Sneg32": Sneg32,
                "Ident": Ident,
                "S128": S128,
            }
        )

    res = run_bass_kernel_spmd(nc, in_maps, list(range(NCORES)))

    o_b = np.empty((B, OUT), dtype=np.float32)
    for c in range(NCORES):
        s, r = divmod(c, JSPLIT)
        ob_c = np.asarray(res.results[c]["ob"])  # [128 o_local, JBLK]
        o_b[r * JBLK : (r + 1) * JBLK, s * OH : (s + 1) * OH] = ob_c.T

    return np.concatenate([x.astype(np.float32), o_b], axis=1)
